# revision 70
# baseline (speedup 1.0000x reference)
"""Cross-attention (B=4, T=S=1024, C=1024, H=16, D=64) on 8 trn2 NeuronCores.

Sharding: core c handles batch b=c//2, query half hf=c%2 (512 q-rows).
NO collectives: each core computes the FULL k/v for its batch (the extra
PE time replaces the ~225us AllGather the collective cost model charges).
All activations stay channel-major on chip; the host transposes per-core
inputs/outputs.

All matmuls run in bf16 (PSUM accumulation fp32); elementwise bf16 on DVE
gets the 2x packed mode.  PE executes only the 640 core GEMM matmuls
(k 128, q 64, v 128, scores 128, y 128, out-proj 64) plus 2 K=1 rb
broadcasts for the last two heads; every other auxiliary op is off PE:

  * RMSNorm sum-of-squares: ACT Square per m-tile, running adds on Pool
    (idle in phase 1), then ONE gpsimd partition_all_reduce -> ssum
    broadcast [128,t]; ACT sqrt and a lazily-emitted DVE reciprocal run
    at full partition width so rr needs no further broadcast (the old
    revision's 24 ones-matmuls + 24 K=1 outer products are gone).
  * RMSNorm apply: f = rr*scale + 1 (DVE two-scalar tensor_scalar),
    out = f*raw (DVE).
  * v bias: folded into the out-proj bias on the HOST (softmax rows sum
    to 1, so attn@(v+bv) = attn@v + bv -> bp' = bv@Wp + bp); the v-chain
    PSUM->SBUF move is a bare ACT copy.
  * softmax 1/Z: the v-augmentation column is memset to 16.0 so row 64
    of y_aug is 16*Z and the DVE reciprocal yields rb=1/(16Z) directly.
    rb is partition-broadcast via a DRAM round-trip (SBUF row -> DRAM ->
    partition_broadcast DMA back, ~4.5us on otherwise-idle DMA engines,
    hazard-tracked by a DRAM tile pool); heads 14/15 use a K=1 ones-row
    matmul + ACT copy instead so the out-projection isn't held up.
  * attn_mean accB[s,j,t] += exp*rb, j-split DVE(0:WSPLIT)/Pool(WSPLIT:8),
    software-pipelined by TWO heads to ride out the rb DMA latency
    (y PSUM bufs=2 tolerates this: y_ps(h) frees mid-iteration h+2).

Schedule: the attention window is elementwise-bound (~4.7us/head: DVE
accB+recip+yT ~4.6, ACT 4 exp chunks ~4.2, Pool accB ~4.3), so PE filler
rides inside it: the v jh=1 chains on head pairs 0-3 and two incremental
out-proj chains (m=0,1, one k-step per head from pair 4) on the v PSUM
banks, leaving a 52-matmul out-proj tail.  Priming (sc/exp of pairs 0-1)
interleaves with the v jh=0 chains.  The first k projection emits its
first six m-chains k-major across 6 PSUM banks so the per-k encoder DMAs
keep PE saturated from ~3.5us; wk arrives m-sliced ([128,m,k,128] host
layout) and small bias/scale DMAs ride between the e-tile DMAs.

TimelineSim 172631 ns (baseline 181514).  Measured rel err ~1.2e-2 vs
the fp32 reference (limit 2e-2).
"""

import numpy as np
import ml_dtypes

import concourse.bacc as bacc
import concourse.bass_isa as bass_isa
import concourse.mybir as mybir
import concourse.tile as tile
from concourse.bass_utils import run_bass_kernel_spmd

F32 = mybir.dt.float32
F32R = mybir.dt.float32r
BF16 = mybir.dt.bfloat16
AF = mybir.ActivationFunctionType
ALU = mybir.AluOpType
RED = bass_isa.ReduceOp

B, T, S, C, H = 4, 1024, 1024, 1024, 16
D = C // H            # 64
TN = 512              # per-core q rows
KT = 8                # contraction tiles (C/128)
MT = 8                # output-channel tiles
ST = 8                # s tiles (S/128)
DA = D + 1            # augmented head width (16.0 column at 64)
WSPLIT = 6            # attn-mean j-tiles on DVE (rest on Pool)


def build():
    nc = bacc.Bacc("TRN2", target_bir_lowering=False, debug=False, num_devices=8)

    xT_d = nc.dram_tensor("xT", [C, TN], BF16, kind="ExternalInput")
    encT_d = nc.dram_tensor("encT", [C, S], BF16, kind="ExternalInput")
    wkr_d = nc.dram_tensor("wkr", [128, MT, KT, 128], BF16, kind="ExternalInput")
    wq_d = nc.dram_tensor("wq", [C, C], BF16, kind="ExternalInput")
    wv_d = nc.dram_tensor("wv", [C, C], BF16, kind="ExternalInput")
    wp_d = nc.dram_tensor("wp", [C, C], BF16, kind="ExternalInput")
    bq_d = nc.dram_tensor("bq", [128, MT], F32, kind="ExternalInput")
    bk_d = nc.dram_tensor("bk", [128, MT], F32, kind="ExternalInput")
    bp_d = nc.dram_tensor("bp", [128, MT], F32, kind="ExternalInput")
    qs_d = nc.dram_tensor("qs", [128, MT], F32, kind="ExternalInput")
    ks_d = nc.dram_tensor("ks", [128, MT], F32, kind="ExternalInput")

    yT_o = nc.dram_tensor("youtT", [C, TN], BF16, kind="ExternalOutput")
    am_o = nc.dram_tensor("ameanT", [S, TN], BF16, kind="ExternalOutput")

    with tile.TileContext(nc) as tc:
        with (
            tc.tile_pool(name="const", bufs=1) as cst,
            tc.tile_pool(name="wk", bufs=1) as wk_pool,
            tc.tile_pool(name="qt", bufs=1) as qt_pool,
            tc.tile_pool(name="kt", bufs=1) as kt_pool,
            tc.tile_pool(name="vt", bufs=1) as vt_pool,
            tc.tile_pool(name="yt", bufs=1) as yt_pool,
            tc.tile_pool(name="am", bufs=1) as am_pool,
            tc.tile_pool(name="wp", bufs=1) as wp_pool,
        ):
            eps_t = cst.tile([128, 1], F32)
            nc.vector.memset(eps_t[:], 1e-6)
            ones_row = cst.tile([1, 128], BF16)
            nc.vector.memset(ones_row[:], 1.0)

            bq_sb = cst.tile([128, MT], F32)
            bk_sb = cst.tile([128, MT], F32)
            bp_sb = cst.tile([128, MT], F32)
            qs_sb = cst.tile([128, MT], F32)
            ks_sb = cst.tile([128, MT], F32)

            wk_sb = wk_pool.tile([128, MT, KT, 128], BF16, name="wk_sb")
            qT = [qt_pool.tile([128, TN], BF16, tag=f"qT{m}", name=f"qT{m}")
                  for m in range(MT)]
            kT = [kt_pool.tile([128, 2, TN], BF16, tag=f"kT{m}", name=f"kT{m}")
                  for m in range(MT)]
            vA = [vt_pool.tile([128, H, DA], BF16, tag=f"vA{j}", name=f"vA{j}")
                  for j in range(ST)]
            yT = [yt_pool.tile([128, TN], BF16, tag=f"yT{k}", name=f"yT{k}")
                  for k in range(KT)]
            accB = am_pool.tile([128, ST, TN], BF16, name="accB")
            wp_sb = wp_pool.tile([128, KT, C], BF16, name="wp_sb")

            # ================= phase 1: projections =================
            with (
                tc.tile_pool(name="p1_in", bufs=1) as p1_in,
                tc.tile_pool(name="p1_w", bufs=2) as p1_w,
            ):
                e_sb = p1_in.tile([128, KT, 2, TN], BF16, name="e_sb")
                x_sb = p1_in.tile([128, KT, TN], BF16, name="x_sb")

                # DMA order = issue order.  wk m=0 slice (729ns) + e0 gate
                # the first matmul at ~3us; the small bias/scale DMAs ride
                # between the e-tile transfers (HWDGE overlaps the bus).
                def e_dma(k, hf2):
                    nc.sync.dma_start(
                        e_sb[:, k, hf2, :],
                        encT_d.ap()[k * 128 : (k + 1) * 128,
                                    hf2 * TN : (hf2 + 1) * TN],
                    )

                # the k0 projection reads only the hf=0 half of each e
                # tile, so the halves load separately: hf0 k-tiles arrive
                # at twice the rate during the paced k-major phase
                nc.sync.dma_start(wk_sb[:, 0], wkr_d.ap()[:, 0])
                e_dma(0, 0)
                nc.sync.dma_start(bk_sb[:], bk_d.ap())
                small = [(ks_sb, ks_d), (bq_sb, bq_d), (qs_sb, qs_d),
                         (bp_sb, bp_d)]
                for k in range(1, KT):
                    # wk m-slices ride ahead of the e tiles so the k-major
                    # interleaved chains m0-5 have weights from ~4us
                    if k < 6:
                        nc.sync.dma_start(wk_sb[:, k], wkr_d.ap()[:, k])
                    e_dma(k, 0)
                    if small:
                        sb, d = small.pop(0)
                        nc.sync.dma_start(sb[:], d.ap())
                for m in range(6, MT):
                    nc.sync.dma_start(wk_sb[:, m], wkr_d.ap()[:, m])
                for k in range(KT):
                    e_dma(k, 1)
                nc.sync.dma_start(
                    x_sb[:], xT_d.ap().rearrange("(k p) t -> p k t", p=128)
                )
                wq_sb = p1_w.tile([128, KT, C], BF16, tag="W", name="wq_sb")
                nc.sync.dma_start(
                    wq_sb[:], wq_d.ap().rearrange("(k p) c -> p k c", p=128)
                )
                wv_sb = p1_w.tile([128, KT, C], BF16, tag="W", name="wv_sb")
                nc.sync.dma_start(
                    wv_sb[:], wv_d.ap().rearrange("(k p) c -> p k c", p=128)
                )
                nc.sync.dma_start(
                    wp_sb[:], wp_d.ap().rearrange("(k p) c -> p k c", p=128)
                )

                def proj_tail(ps, m, bias_sb, acc, raws):
                    """Bias->raw (DVE), square (ACT), running sum (Pool)."""
                    raw = p1_raw.tile([128, TN], BF16, tag=f"raw{m}")
                    nc.vector.tensor_scalar_add(
                        raw[:], ps[:], bias_sb[:, m : m + 1]
                    )
                    sq = p1_sq.tile([128, TN], F32R, tag="sq")
                    nc.scalar.activation(
                        sq[:], ps[:], AF.Square, bias=bias_sb[:, m : m + 1]
                    )
                    raws.append(raw)
                    if m == 0:
                        proj_tail.sq0 = sq
                    elif m == 1:
                        nc.gpsimd.tensor_add(acc[:], proj_tail.sq0[:], sq[:])
                    else:
                        nc.gpsimd.tensor_add(acc[:], acc[:], sq[:])

                def proj_rms(acc):
                    """partition_all_reduce -> sqrt, both [128,TN].  The
                    DVE reciprocal is deferred to proj_B so it doesn't
                    head-block the in-order DVE queue while sqrt is still
                    pending."""
                    ssb = p1_r.tile([128, TN], F32R, tag="ssb", bufs=1)
                    nc.gpsimd.partition_all_reduce(ssb[:], acc[:], 128, RED.add)
                    rms = p1_r.tile([128, TN], F32, tag="rms", bufs=2)
                    nc.scalar.activation(
                        rms[:], ssb[:], AF.Sqrt, scale=1.0 / C,
                        bias=eps_t[:, 0:1],
                    )
                    return rms

                def proj_A(w_ap, in_ap, bias_sb, paced=False):
                    """Projection matmuls + bias/sum-of-squares tail.

                    paced=True emits the first four m-chains k-major across
                    4 PSUM banks so per-k input DMAs keep PE saturated.
                    """
                    acc = p1_r.tile([128, TN], F32R, tag="acc")
                    raws = []
                    if paced:
                        for lo, n in ((0, 6), (6, 2)):
                            pss = [p1_ps.tile([128, TN], F32, tag="proj",
                                              name=f"pacedps{i}")
                                   for i in range(n)]
                            for k in range(KT):
                                for i, ps in enumerate(pss):
                                    nc.tensor.matmul(
                                        ps[:],
                                        w_ap(lo + i, k),
                                        in_ap(k),
                                        start=(k == 0),
                                        stop=(k == KT - 1),
                                    )
                            for i, ps in enumerate(pss):
                                proj_tail(ps, lo + i, bias_sb, acc, raws)
                    else:
                        for m in range(MT):
                            ps = p1_ps.tile([128, TN], F32, tag="proj")
                            for k in range(KT):
                                nc.tensor.matmul(
                                    ps[:],
                                    w_ap(m, k),
                                    in_ap(k),
                                    start=(k == 0),
                                    stop=(k == KT - 1),
                                )
                            proj_tail(ps, m, bias_sb, acc, raws)
                    return raws, proj_rms(acc)

                def proj_B(raws, rms, scale_sb, out_ap, ms=None):
                    """RMSNorm+residual apply: out = (scale*rr + 1) * raw.

                    All ops on DVE (two-scalar tensor_scalar for f) —
                    ACT's in-order queue is the scarce phase-1 resource
                    (squares + v copies + priming exps).
                    """
                    rrB = p1_r.tile([128, TN], F32R, tag="rr")
                    with nc.allow_low_precision(reason="rms rsqrt"):
                        nc.vector.reciprocal(rrB[:], rms[:])
                    for m in (range(MT) if ms is None else ms):
                        f = p1_r.tile([128, TN], F32R, tag="f")
                        nc.vector.tensor_scalar(
                            f[:], rrB[:], scale_sb[:, m : m + 1], 1.0,
                            ALU.mult, ALU.add,
                        )
                        nc.vector.tensor_mul(out_ap(m), f[:], raws[m][:])

                def v_chain(st, jh, pool):
                    # bv is folded into the out-proj bias on the host
                    # (softmax rows sum to 1: attn@(v+bv) = attn@v + bv),
                    # so the PSUM->SBUF move is a pure ACT copy and the
                    # in-order DVE queue never gates the v chains.
                    ps = pool.tile([128, TN], F32, tag="proj")
                    for k in range(KT):
                        nc.tensor.matmul(
                            ps[:],
                            e_sb[:, k, st // 4,
                                 (st % 4) * 128 : (st % 4 + 1) * 128],
                            wv_sb[:, k, jh * 512 : (jh + 1) * 512],
                            start=(k == 0),
                            stop=(k == KT - 1),
                        )
                    nc.scalar.copy(
                        vA[st][:, jh * 8 : (jh + 1) * 8, 0:D],
                        ps[:].rearrange("p (h d) -> p h d", h=8),
                    )

                # qk projections; the jh=0 v chains share the ps pool and
                # carry the sc/exp priming of head pairs 0-1.
                with (
                    tc.tile_pool(name="p1_raw", bufs=2) as p1_raw,
                    tc.tile_pool(name="p1_sq", bufs=2) as p1_sq,
                    tc.tile_pool(name="p1_r", bufs=2) as p1_r,
                    tc.tile_pool(name="p1_ps", bufs=6, space="PSUM") as p1_ps,
                ):
                    k_w = lambda m, k: wk_sb[:, m, k, :]
                    k_in = [lambda k, hf2=hf2: e_sb[:, k, hf2, :]
                            for hf2 in range(2)]
                    k_out = [lambda m, hf2=hf2: kT[m][:, hf2, :]
                             for hf2 in range(2)]
                    q_w = lambda m, k: wq_sb[:, k, m * 128 : (m + 1) * 128]
                    rk0 = proj_A(k_w, k_in[0], bk_sb, paced=True)
                    rk1 = proj_A(k_w, k_in[1], bk_sb)
                    proj_B(*rk0, ks_sb, k_out[0])
                    rq = proj_A(q_w, lambda k: x_sb[:, k, :], bq_sb)
                    # qB ahead of kB1: the jj2<2 priming chunks read only
                    # the hf0 half of kT, so qT is the scarcer input
                    proj_B(*rq, qs_sb, lambda m: qT[m][:, :])
                    proj_B(*rk1, ks_sb, k_out[1])

                # score PSUM pool spans v-proj + attention so the sc/exp
                # chain of early head-pairs overlaps the v matmuls
                with (
                    tc.tile_pool(name="p2_e", bufs=3) as p2_e,
                    tc.tile_pool(name="p2_rb", bufs=3) as p2_rb,
                    tc.tile_pool(name="rbd", bufs=3, space="DRAM") as rbd,
                    tc.tile_pool(name="p2_sc", bufs=2, space="PSUM") as p2_sc,
                    tc.tile_pool(name="p1_vps", bufs=2, space="PSUM") as p1_vps,
                ):
                    exp_tiles = {}

                    def sc_exp_chunk(p, hh, jj2):
                        """Two K=64 score matmuls + one 2-bank exp."""
                        if p not in exp_tiles:
                            exp_tiles[p] = p2_e.tile(
                                [128, 2, ST, TN], BF16, tag="exp",
                                name=f"expT{p}",
                            )
                        expT = exp_tiles[p]
                        base = 64 * hh
                        sc = p2_sc.tile([128, 2, TN], F32, tag="sc")
                        for u in range(2):
                            j = 2 * jj2 + u
                            nc.tensor.matmul(
                                sc[:, u, :],
                                kT[p][base : base + 64, j // 4,
                                      (j % 4) * 128 : (j % 4 + 1) * 128],
                                qT[p][base : base + 64, :],
                                start=True,
                                stop=True,
                            )
                        nc.scalar.activation(
                            expT[:, hh, 2 * jj2 : 2 * jj2 + 2, :],
                            sc[:],
                            AF.Exp,
                            scale=float(D) ** -0.5,
                        )

                    # v jh=0 chains with sc/exp of pairs 0-1 interleaved so
                    # attention is primed when the first-half v completes
                    PRIMED = 2
                    prime_units = [(p, hh, jj2) for p in range(PRIMED)
                                   for hh in range(2) for jj2 in range(4)]
                    # one prime chunk per chain after the first six: the
                    # early-chain v copies must not queue behind priming
                    # exps on ACT (exps wait on qT, which lands last)
                    ui = 0
                    for st in range(ST):
                        nc.gpsimd.memset(vA[st][:, :, D:DA], 16.0)
                        v_chain(st, 0, p1_vps)
                        if ui < len(prime_units):
                            sc_exp_chunk(*prime_units[ui])
                            ui += 1
                    # one v jh=1 chain ahead of the priming tail: its
                    # matmuls are dependency-free and fill the PE hole
                    # while the q rms chain finishes (the tail's sc
                    # matmuls park on qT in the 4-deep wait queue)
                    v_chain(0, 1, p1_vps)
                    while ui < len(prime_units):
                        sc_exp_chunk(*prime_units[ui])
                        ui += 1

                    # ============= phase 2: attention =============
                    def attn_acc(h, hh, expT, rb_sb):
                        """attn_mean: accB[:, j, :] += exp_h[:, j, :] * rb,
                        j-split DVE(0:WSPLIT)/Pool(WSPLIT:8)."""
                        rbb = rb_sb[:].rearrange("p (a t) -> p a t", a=1)
                        wd, wp_ = WSPLIT, ST - WSPLIT
                        if h == 0:
                            nc.vector.tensor_mul(
                                accB[:, 0:wd, :], expT[:, hh, 0:wd, :],
                                rbb.broadcast_to((128, wd, TN)),
                            )
                            nc.gpsimd.tensor_mul(
                                accB[:, wd:ST, :], expT[:, hh, wd:ST, :],
                                rbb.broadcast_to((128, wp_, TN)),
                            )
                        else:
                            nc.vector.tensor_mul(
                                expT[:, hh, 0:wd, :], expT[:, hh, 0:wd, :],
                                rbb.broadcast_to((128, wd, TN)),
                            )
                            nc.vector.tensor_add(
                                accB[:, 0:wd, :], accB[:, 0:wd, :],
                                expT[:, hh, 0:wd, :],
                            )
                            nc.gpsimd.tensor_mul(
                                expT[:, hh, wd:ST, :], expT[:, hh, wd:ST, :],
                                rbb.broadcast_to((128, wp_, TN)),
                            )
                            nc.gpsimd.tensor_add(
                                accB[:, wd:ST, :], accB[:, wd:ST, :],
                                expT[:, hh, wd:ST, :],
                            )

                    # software-pipelined by TWO heads: rb travels through a
                    # DRAM round-trip (SBUF row -> DRAM -> partition-bcast
                    # back, ~4.5us latency on otherwise-idle DMA engines),
                    # so head h's yT/attn_acc run during head h+2.  The
                    # last two heads broadcast via a K=1 PE matmul instead
                    # (~1us) so the out-projection isn't held up.  The
                    # v jh=1 chains ride along pairs 0-3 as PE filler
                    # (their outputs are first read at head 8).
                    def finish_head(fin):
                        h, hh, mt, base, expT, y_ps, rb_sb = fin
                        # yT[c-tile mt, base:base+64] = 16 * y_aug * rb
                        nc.vector.scalar_tensor_tensor(
                            yT[mt][base : base + 64, :],
                            y_ps[0:64, :],
                            16.0,
                            rb_sb[0:64, :],
                            ALU.mult,
                            ALU.mult,
                        )
                        attn_acc(h, hh, expT, rb_sb)

                    pending = []
                    chunk_q = [(p_, hh_, jj2_) for p_ in range(PRIMED, H // 2)
                               for hh_ in range(2) for jj2_ in range(4)]
                    ci = [0]
                    with tc.tile_pool(name="p2_y", bufs=2, space="PSUM") as p2_y:
                        for p in range(H // 2):
                            mt = p
                            for hh in range(2):
                                h = 2 * p + hh
                                base = 64 * hh
                                # finish head h-2 BEFORE y_ps(h) is
                                # allocated (bufs=2 WAR rotation)
                                if len(pending) >= 2:
                                    finish_head(pending.pop(0))
                                # emit this head's remaining sc chunks (two
                                # were pre-issued as lookahead last head)
                                while ci[0] < max(0, (h + 1 - 2 * PRIMED)) * 4:
                                    sc_exp_chunk(*chunk_q[ci[0]])
                                    ci[0] += 1
                                expT = exp_tiles[p]
                                y_ps = p2_y.tile([DA, TN], F32, tag="y")
                                for j in range(ST):
                                    nc.tensor.matmul(
                                        y_ps[:],
                                        vA[j][:, h, :],
                                        expT[:, hh, j, :],
                                        start=(j == 0),
                                        stop=(j == ST - 1),
                                    )
                                if p < 4 and 2 * p + hh + 1 < ST:
                                    v_chain(2 * p + hh + 1, 1, p1_vps)
                                elif hh == 0 and p == 4:
                                    # incremental out-proj chains m=0,1 on
                                    # the two now-idle v PSUM banks: their
                                    # k=p-4 step rides each later pair, so
                                    # only k>=4 remains after the last head
                                    op_ps = [
                                        p1_vps.tile([128, TN], F32,
                                                    tag="proj",
                                                    name=f"op{i}_ps")
                                        for i in range(2)
                                    ]
                                if p >= 4 and 2 * (p - 4) + hh <= 5:
                                    # both chains advance one k-step per
                                    # head: k=2(p-4)+hh needs yT[k], i.e.
                                    # head 2k+1 finished -- true for k<=5
                                    # under the depth-2 finish pipeline
                                    ok = 2 * (p - 4) + hh
                                    for i in range(2):
                                        nc.tensor.matmul(
                                            op_ps[i][:],
                                            wp_sb[:, ok,
                                                  i * 128 : (i + 1) * 128],
                                            yT[ok][:],
                                            start=(ok == 0),
                                            stop=False,
                                        )
                                # row 64 of y_ps is 16*Z -> rb = 1/(16Z)
                                recip = p2_rb.tile([1, TN], BF16, tag="recip")
                                with nc.allow_low_precision(reason="softmax 1/Z"):
                                    nc.vector.reciprocal(recip[:], y_ps[64:65, :])
                                rb_sb = p2_rb.tile([128, TN], BF16, tag="rbsb")
                                if h < H - 2:
                                    rb_d = rbd.tile([TN], BF16, tag="rbd")
                                    nc.sync.dma_start(rb_d[:], recip[:])
                                    nc.sync.dma_start(
                                        rb_sb[:],
                                        rb_d[:].partition_broadcast(128),
                                    )
                                else:
                                    rb_ps = p2_sc.tile([128, 2, TN], F32,
                                                       tag="sc",
                                                       name=f"rbps{hh}")
                                    nc.tensor.matmul(
                                        rb_ps[:, 0, :], ones_row[:], recip[:],
                                        start=True, stop=True,
                                    )
                                    nc.scalar.activation(
                                        rb_sb[:], rb_ps[:, 0, :], AF.Copy
                                    )
                                pending.append(
                                    (h, hh, mt, base, expT, y_ps, rb_sb)
                                )
                                # 2-chunk lookahead into the next head: its
                                # 4 matmuls fit the 4-deep PE wait queue,
                                # so the exps cook during this head's tail
                                la = min(len(chunk_q),
                                         max(0, (h + 2 - 2 * PRIMED)) * 4 - 2)
                                while ci[0] < la:
                                    sc_exp_chunk(*chunk_q[ci[0]])
                                    ci[0] += 1
                        while pending:
                            finish_head(pending.pop(0))
                        nc.sync.dma_start(
                            am_o.ap().rearrange("(j p) t -> p j t", p=128),
                            accB[:],
                        )

                        # ===== phase 3: output projection =====
                        # emitted inside the attention pool scope (pool
                        # close would drain-barrier on the final attn_acc)
                        # and accumulated in the sc PSUM pool's banks.
                        with tc.tile_pool(name="p3_o", bufs=3) as p3_o:
                            # finish the incremental m=0,1 chains (k>=6)
                            for k in range(6, KT):
                                for i in range(2):
                                    nc.tensor.matmul(
                                        op_ps[i][:],
                                        wp_sb[:, k, i * 128 : (i + 1) * 128],
                                        yT[k][:],
                                        start=False,
                                        stop=(k == KT - 1),
                                    )
                            for i in range(2):
                                yo = p3_o.tile([128, TN], BF16, tag="yo_sb")
                                nc.scalar.activation(
                                    yo[:], op_ps[i][:], AF.Identity,
                                    bias=bp_sb[:, i : i + 1],
                                )
                                nc.sync.dma_start(
                                    yT_o.ap()[i * 128 : (i + 1) * 128, :],
                                    yo[:],
                                )
                            for m in range(2, MT):
                                ps = p2_sc.tile([128, TN], F32, tag="sc",
                                                name=f"yo_ps{m}")
                                for k in range(KT):
                                    nc.tensor.matmul(
                                        ps[:],
                                        wp_sb[:, k, m * 128 : (m + 1) * 128],
                                        yT[k][:],
                                        start=(k == 0),
                                        stop=(k == KT - 1),
                                    )
                                yo = p3_o.tile([128, TN], BF16, tag="yo_sb")
                                # bias add on ACT
                                nc.scalar.activation(
                                    yo[:], ps[:], AF.Identity,
                                    bias=bp_sb[:, m : m + 1],
                                )
                                nc.sync.dma_start(
                                    yT_o.ap()[m * 128 : (m + 1) * 128, :],
                                    yo[:],
                                )

    nc.compile()
    return nc


_NC_CACHE = None


def _get_nc():
    global _NC_CACHE
    if _NC_CACHE is None:
        _NC_CACHE = build()
    return _NC_CACHE


BF = ml_dtypes.bfloat16


def make_in_maps(x, encoder_output, Wq, bq, Wk, bk, Wv, bv, q_scale, k_scale,
                 Wp, bp):
    x = np.asarray(x, np.float32)
    enc = np.asarray(encoder_output, np.float32)
    Wk_b = np.asarray(Wk, np.float32).astype(BF)
    # m-sliced wk layout: wkr[p, m, k, c] = Wk[k*128+p, m*128+c]
    wkr = np.ascontiguousarray(
        Wk_b.reshape(KT, 128, MT, 128).transpose(1, 2, 0, 3)
    )
    Wq_b = np.ascontiguousarray(np.asarray(Wq, np.float32)).astype(BF)
    Wv_b = np.ascontiguousarray(np.asarray(Wv, np.float32)).astype(BF)
    Wp_b = np.ascontiguousarray(np.asarray(Wp, np.float32)).astype(BF)
    bq_t = np.ascontiguousarray(np.asarray(bq, np.float32).reshape(MT, 128).T)
    bk_t = np.ascontiguousarray(np.asarray(bk, np.float32).reshape(MT, 128).T)
    # bv folded into the out-proj bias: attn@(v+bv) = attn@v + bv
    bp_f = (np.asarray(bv, np.float32) @ np.asarray(Wp, np.float32)
            + np.asarray(bp, np.float32))
    bp_t = np.ascontiguousarray(bp_f.reshape(MT, 128).T)
    qs_t = np.ascontiguousarray(
        np.asarray(q_scale, np.float32).reshape(MT, 128).T
    )
    ks_t = np.ascontiguousarray(
        np.asarray(k_scale, np.float32).reshape(MT, 128).T
    )

    encT_b = [np.ascontiguousarray(enc[b].T).astype(BF) for b in range(B)]

    in_maps = []
    for c in range(8):
        b, hf = c // 2, c % 2
        xT = np.ascontiguousarray(x[b, hf * TN : (hf + 1) * TN, :].T).astype(BF)
        in_maps.append(
            dict(xT=xT, encT=encT_b[b], wkr=wkr, wq=Wq_b, wv=Wv_b, wp=Wp_b,
                 bq=bq_t, bk=bk_t, bp=bp_t, qs=qs_t, ks=ks_t)
        )
    return in_maps


def kernel(x, encoder_output, Wq, bq, Wk, bk, Wv, bv, q_scale, k_scale, Wp, bp,
           _trace=False):
    in_maps = make_in_maps(x, encoder_output, Wq, bq, Wk, bk, Wv, bv, q_scale,
                           k_scale, Wp, bp)
    nc = _get_nc()
    res = run_bass_kernel_spmd(nc, in_maps, core_ids=list(range(8)), trace=_trace)

    y = np.empty((B, T, C), np.float32)
    amean = np.empty((B, T, S), np.float32)
    for c in range(8):
        b, hf = c // 2, c % 2
        r = res.results[c]
        y[b, hf * TN : (hf + 1) * TN, :] = r["youtT"].astype(np.float32).T
        amean[b, hf * TN : (hf + 1) * TN, :] = (
            r["ameanT"].astype(np.float32).T
        )
    if _trace:
        kernel.last_exec_time_ns = res.exec_time_ns
        kernel.last_results = res
    return y, amean


# revision 71
# speedup vs baseline: 1.0050x; 1.0050x over previous
"""Cross-attention (B=4, T=S=1024, C=1024, H=16, D=64) on 8 trn2 NeuronCores.

Sharding: core c handles batch b=c//2, query half hf=c%2 (512 q-rows).
NO collectives: each core computes the FULL k/v for its batch (the extra
PE time replaces the ~225us AllGather the collective cost model charges).
All activations stay channel-major on chip; the host transposes per-core
inputs/outputs.

All matmuls run in bf16 (PSUM accumulation fp32); elementwise bf16 on DVE
gets the 2x packed mode.  PE executes only the 640 core GEMM matmuls
(k 128, q 64, v 128, scores 128, y 128, out-proj 64) plus 2 K=1 rb
broadcasts for the last two heads; every other auxiliary op is off PE:

  * RMSNorm sum-of-squares: ACT Square per m-tile, running adds on Pool
    (idle in phase 1), then ONE gpsimd partition_all_reduce -> ssum
    broadcast [128,t]; ACT sqrt and a lazily-emitted DVE reciprocal run
    at full partition width so rr needs no further broadcast (the old
    revision's 24 ones-matmuls + 24 K=1 outer products are gone).
  * RMSNorm apply: f = rr*scale + 1 (DVE two-scalar tensor_scalar),
    out = f*raw (DVE).
  * v bias: folded into the out-proj bias on the HOST (softmax rows sum
    to 1, so attn@(v+bv) = attn@v + bv -> bp' = bv@Wp + bp); the v-chain
    PSUM->SBUF move is a bare ACT copy.
  * softmax 1/Z: the v-augmentation column is memset to 16.0 so row 64
    of y_aug is 16*Z and the DVE reciprocal yields rb=1/(16Z) directly.
    rb is partition-broadcast via a DRAM round-trip (SBUF row -> DRAM ->
    partition_broadcast DMA back, ~4.5us on otherwise-idle DMA engines,
    hazard-tracked by a DRAM tile pool); heads 14/15 use a K=1 ones-row
    matmul + ACT copy instead so the out-projection isn't held up.
  * attn_mean accB[s,j,t] += exp*rb, j-split DVE(0:WSPLIT)/Pool(WSPLIT:8),
    software-pipelined by TWO heads to ride out the rb DMA latency
    (y PSUM bufs=2 tolerates this: y_ps(h) frees mid-iteration h+2).

Schedule: the attention window is elementwise-bound (~4.7us/head: DVE
accB+recip+yT ~4.6, ACT 4 exp chunks ~4.2, Pool accB ~4.3), so PE filler
rides inside it: the v jh=1 chains on head pairs 0-3 and two incremental
out-proj chains (m=0,1, one k-step per head from pair 4) on the v PSUM
banks, leaving a 52-matmul out-proj tail.  Priming (sc/exp of pairs 0-1)
interleaves with the v jh=0 chains.  The first k projection emits its
first six m-chains k-major across 6 PSUM banks so the per-k encoder DMAs
keep PE saturated from ~3.5us; wk arrives m-sliced ([128,m,k,128] host
layout) and small bias/scale DMAs ride between the e-tile DMAs.

TimelineSim 172631 ns (baseline 181514).  Measured rel err ~1.2e-2 vs
the fp32 reference (limit 2e-2).
"""

import numpy as np
import ml_dtypes

import concourse.bacc as bacc
import concourse.bass_isa as bass_isa
import concourse.mybir as mybir
import concourse.tile as tile
from concourse.bass_utils import run_bass_kernel_spmd

F32 = mybir.dt.float32
F32R = mybir.dt.float32r
BF16 = mybir.dt.bfloat16
AF = mybir.ActivationFunctionType
ALU = mybir.AluOpType
RED = bass_isa.ReduceOp

B, T, S, C, H = 4, 1024, 1024, 1024, 16
D = C // H            # 64
TN = 512              # per-core q rows
KT = 8                # contraction tiles (C/128)
MT = 8                # output-channel tiles
ST = 8                # s tiles (S/128)
DA = D + 1            # augmented head width (16.0 column at 64)
WSPLIT = 6            # attn-mean j-tiles on DVE (rest on Pool)


def build():
    nc = bacc.Bacc("TRN2", target_bir_lowering=False, debug=False, num_devices=8)

    xT_d = nc.dram_tensor("xT", [C, TN], BF16, kind="ExternalInput")
    encT_d = nc.dram_tensor("encT", [C, S], BF16, kind="ExternalInput")
    wkr_d = nc.dram_tensor("wkr", [128, MT, KT, 128], BF16, kind="ExternalInput")
    wq_d = nc.dram_tensor("wq", [C, C], BF16, kind="ExternalInput")
    wv_d = nc.dram_tensor("wv", [C, C], BF16, kind="ExternalInput")
    wp_d = nc.dram_tensor("wp", [C, C], BF16, kind="ExternalInput")
    bq_d = nc.dram_tensor("bq", [128, MT], F32, kind="ExternalInput")
    bk_d = nc.dram_tensor("bk", [128, MT], F32, kind="ExternalInput")
    bp_d = nc.dram_tensor("bp", [128, MT], F32, kind="ExternalInput")
    qs_d = nc.dram_tensor("qs", [128, MT], F32, kind="ExternalInput")
    ks_d = nc.dram_tensor("ks", [128, MT], F32, kind="ExternalInput")

    yT_o = nc.dram_tensor("youtT", [C, TN], BF16, kind="ExternalOutput")
    am_o = nc.dram_tensor("ameanT", [S, TN], BF16, kind="ExternalOutput")

    with tile.TileContext(nc) as tc:
        with (
            tc.tile_pool(name="const", bufs=1) as cst,
            tc.tile_pool(name="wk", bufs=1) as wk_pool,
            tc.tile_pool(name="qt", bufs=1) as qt_pool,
            tc.tile_pool(name="kt", bufs=1) as kt_pool,
            tc.tile_pool(name="vt", bufs=1) as vt_pool,
            tc.tile_pool(name="yt", bufs=1) as yt_pool,
            tc.tile_pool(name="am", bufs=1) as am_pool,
            tc.tile_pool(name="wp", bufs=1) as wp_pool,
        ):
            eps_t = cst.tile([128, 1], F32)
            nc.vector.memset(eps_t[:], 1e-6)
            ones_row = cst.tile([1, 128], BF16)
            nc.vector.memset(ones_row[:], 1.0)

            bq_sb = cst.tile([128, MT], F32)
            bk_sb = cst.tile([128, MT], F32)
            bp_sb = cst.tile([128, MT], F32)
            qs_sb = cst.tile([128, MT], F32)
            ks_sb = cst.tile([128, MT], F32)

            wk_sb = wk_pool.tile([128, MT, KT, 128], BF16, name="wk_sb")
            qT = [qt_pool.tile([128, TN], BF16, tag=f"qT{m}", name=f"qT{m}")
                  for m in range(MT)]
            kT = [kt_pool.tile([128, 2, TN], BF16, tag=f"kT{m}", name=f"kT{m}")
                  for m in range(MT)]
            vA = [vt_pool.tile([128, H, DA], BF16, tag=f"vA{j}", name=f"vA{j}")
                  for j in range(ST)]
            yT = [yt_pool.tile([128, TN], BF16, tag=f"yT{k}", name=f"yT{k}")
                  for k in range(KT)]
            accB = am_pool.tile([128, ST, TN], BF16, name="accB")
            wp_sb = wp_pool.tile([128, KT, C], BF16, name="wp_sb")

            # ================= phase 1: projections =================
            with (
                tc.tile_pool(name="p1_in", bufs=1) as p1_in,
                tc.tile_pool(name="p1_w", bufs=2) as p1_w,
            ):
                e_sb = p1_in.tile([128, KT, 2, TN], BF16, name="e_sb")
                x_sb = p1_in.tile([128, KT, TN], BF16, name="x_sb")

                # DMA order = issue order.  wk m=0 slice (729ns) + e0 gate
                # the first matmul at ~3us; the small bias/scale DMAs ride
                # between the e-tile transfers (HWDGE overlaps the bus).
                def e_dma(k, hf2):
                    nc.sync.dma_start(
                        e_sb[:, k, hf2, :],
                        encT_d.ap()[k * 128 : (k + 1) * 128,
                                    hf2 * TN : (hf2 + 1) * TN],
                    )

                # the k0 projection reads only the hf=0 half of each e
                # tile, so the halves load separately: hf0 k-tiles arrive
                # at twice the rate during the paced k-major phase
                nc.sync.dma_start(wk_sb[:, 0], wkr_d.ap()[:, 0])
                e_dma(0, 0)
                nc.sync.dma_start(bk_sb[:], bk_d.ap())
                small = [(ks_sb, ks_d), (bq_sb, bq_d), (qs_sb, qs_d),
                         (bp_sb, bp_d)]
                for k in range(1, KT):
                    # wk m-slices ride ahead of the e tiles so the k-major
                    # interleaved chains m0-5 have weights from ~4us
                    if k < 6:
                        nc.sync.dma_start(wk_sb[:, k], wkr_d.ap()[:, k])
                    e_dma(k, 0)
                    if small:
                        sb, d = small.pop(0)
                        nc.sync.dma_start(sb[:], d.ap())
                for m in range(6, MT):
                    nc.sync.dma_start(wk_sb[:, m], wkr_d.ap()[:, m])
                for k in range(KT):
                    e_dma(k, 1)
                nc.sync.dma_start(
                    x_sb[:], xT_d.ap().rearrange("(k p) t -> p k t", p=128)
                )
                wq_sb = p1_w.tile([128, KT, C], BF16, tag="W", name="wq_sb")
                nc.sync.dma_start(
                    wq_sb[:], wq_d.ap().rearrange("(k p) c -> p k c", p=128)
                )
                wv_sb = p1_w.tile([128, KT, C], BF16, tag="W", name="wv_sb")
                nc.sync.dma_start(
                    wv_sb[:], wv_d.ap().rearrange("(k p) c -> p k c", p=128)
                )
                nc.sync.dma_start(
                    wp_sb[:], wp_d.ap().rearrange("(k p) c -> p k c", p=128)
                )

                def proj_tail(ps, m, bias_sb, acc, raws):
                    """Bias->raw (DVE), square (ACT), running sum (Pool)."""
                    raw = p1_raw.tile([128, TN], BF16, tag=f"raw{m}")
                    nc.vector.tensor_scalar_add(
                        raw[:], ps[:], bias_sb[:, m : m + 1]
                    )
                    sq = p1_sq.tile([128, TN], F32R, tag="sq")
                    nc.scalar.activation(
                        sq[:], ps[:], AF.Square, bias=bias_sb[:, m : m + 1]
                    )
                    raws.append(raw)
                    if m == 0:
                        proj_tail.sq0 = sq
                    elif m == 1:
                        nc.gpsimd.tensor_add(acc[:], proj_tail.sq0[:], sq[:])
                    else:
                        nc.gpsimd.tensor_add(acc[:], acc[:], sq[:])

                def proj_rms(acc):
                    """partition_all_reduce -> sqrt, both [128,TN].  The
                    DVE reciprocal is deferred to proj_B so it doesn't
                    head-block the in-order DVE queue while sqrt is still
                    pending."""
                    ssb = p1_r.tile([128, TN], F32R, tag="ssb", bufs=1)
                    nc.gpsimd.partition_all_reduce(ssb[:], acc[:], 128, RED.add)
                    rms = p1_r.tile([128, TN], F32, tag="rms", bufs=2)
                    nc.scalar.activation(
                        rms[:], ssb[:], AF.Sqrt, scale=1.0 / C,
                        bias=eps_t[:, 0:1],
                    )
                    return rms

                def proj_A(w_ap, in_ap, bias_sb, paced=False):
                    """Projection matmuls + bias/sum-of-squares tail.

                    paced=True emits the first four m-chains k-major across
                    4 PSUM banks so per-k input DMAs keep PE saturated.
                    """
                    acc = p1_r.tile([128, TN], F32R, tag="acc")
                    raws = []
                    if paced:
                        for lo, n in ((0, 6), (6, 2)):
                            pss = [p1_ps.tile([128, TN], F32, tag="proj",
                                              name=f"pacedps{i}")
                                   for i in range(n)]
                            for k in range(KT):
                                for i, ps in enumerate(pss):
                                    nc.tensor.matmul(
                                        ps[:],
                                        w_ap(lo + i, k),
                                        in_ap(k),
                                        start=(k == 0),
                                        stop=(k == KT - 1),
                                    )
                            for i, ps in enumerate(pss):
                                proj_tail(ps, lo + i, bias_sb, acc, raws)
                    else:
                        for m in range(MT):
                            ps = p1_ps.tile([128, TN], F32, tag="proj")
                            for k in range(KT):
                                nc.tensor.matmul(
                                    ps[:],
                                    w_ap(m, k),
                                    in_ap(k),
                                    start=(k == 0),
                                    stop=(k == KT - 1),
                                )
                            proj_tail(ps, m, bias_sb, acc, raws)
                    return raws, proj_rms(acc)

                def proj_B(raws, rms, scale_sb, out_ap, ms=None):
                    """RMSNorm+residual apply: out = (scale*rr + 1) * raw.

                    All ops on DVE (two-scalar tensor_scalar for f) —
                    ACT's in-order queue is the scarce phase-1 resource
                    (squares + v copies + priming exps).
                    """
                    rrB = p1_r.tile([128, TN], F32R, tag="rr")
                    with nc.allow_low_precision(reason="rms rsqrt"):
                        nc.vector.reciprocal(rrB[:], rms[:])
                    for m in (range(MT) if ms is None else ms):
                        f = p1_r.tile([128, TN], F32R, tag="f")
                        nc.vector.tensor_scalar(
                            f[:], rrB[:], scale_sb[:, m : m + 1], 1.0,
                            ALU.mult, ALU.add,
                        )
                        nc.vector.tensor_mul(out_ap(m), f[:], raws[m][:])

                def v_chain(st, jh, pool):
                    # bv is folded into the out-proj bias on the host
                    # (softmax rows sum to 1: attn@(v+bv) = attn@v + bv),
                    # so the PSUM->SBUF move is a pure ACT copy and the
                    # in-order DVE queue never gates the v chains.
                    ps = pool.tile([128, TN], F32, tag="proj")
                    for k in range(KT):
                        nc.tensor.matmul(
                            ps[:],
                            e_sb[:, k, st // 4,
                                 (st % 4) * 128 : (st % 4 + 1) * 128],
                            wv_sb[:, k, jh * 512 : (jh + 1) * 512],
                            start=(k == 0),
                            stop=(k == KT - 1),
                        )
                    nc.scalar.copy(
                        vA[st][:, jh * 8 : (jh + 1) * 8, 0:D],
                        ps[:].rearrange("p (h d) -> p h d", h=8),
                    )

                # qk projections; the jh=0 v chains share the ps pool and
                # carry the sc/exp priming of head pairs 0-1.
                with (
                    tc.tile_pool(name="p1_raw", bufs=2) as p1_raw,
                    tc.tile_pool(name="p1_sq", bufs=2) as p1_sq,
                    tc.tile_pool(name="p1_r", bufs=2) as p1_r,
                    tc.tile_pool(name="p1_ps", bufs=6, space="PSUM") as p1_ps,
                ):
                    k_w = lambda m, k: wk_sb[:, m, k, :]
                    k_in = [lambda k, hf2=hf2: e_sb[:, k, hf2, :]
                            for hf2 in range(2)]
                    k_out = [lambda m, hf2=hf2: kT[m][:, hf2, :]
                             for hf2 in range(2)]
                    q_w = lambda m, k: wq_sb[:, k, m * 128 : (m + 1) * 128]
                    rk0 = proj_A(k_w, k_in[0], bk_sb, paced=True)
                    rk1 = proj_A(k_w, k_in[1], bk_sb)
                    proj_B(*rk0, ks_sb, k_out[0])
                    rq = proj_A(q_w, lambda k: x_sb[:, k, :], bq_sb)
                    # qB ahead of kB1: the jj2<2 priming chunks read only
                    # the hf0 half of kT, so qT is the scarcer input
                    proj_B(*rq, qs_sb, lambda m: qT[m][:, :])
                    proj_B(*rk1, ks_sb, k_out[1])

                # score PSUM pool spans v-proj + attention so the sc/exp
                # chain of early head-pairs overlaps the v matmuls
                with (
                    tc.tile_pool(name="p2_e", bufs=3) as p2_e,
                    tc.tile_pool(name="p2_rb", bufs=3) as p2_rb,
                    tc.tile_pool(name="rbd", bufs=3, space="DRAM") as rbd,
                    tc.tile_pool(name="p2_sc", bufs=2, space="PSUM") as p2_sc,
                    tc.tile_pool(name="p1_vps", bufs=2, space="PSUM") as p1_vps,
                ):
                    exp_tiles = {}

                    def sc_exp_chunk(p, hh, jj2):
                        """Two K=64 score matmuls + one 2-bank exp."""
                        if p not in exp_tiles:
                            exp_tiles[p] = p2_e.tile(
                                [128, 2, ST, TN], BF16, tag="exp",
                                name=f"expT{p}",
                            )
                        expT = exp_tiles[p]
                        base = 64 * hh
                        sc = p2_sc.tile([128, 2, TN], F32, tag="sc")
                        for u in range(2):
                            j = 2 * jj2 + u
                            nc.tensor.matmul(
                                sc[:, u, :],
                                kT[p][base : base + 64, j // 4,
                                      (j % 4) * 128 : (j % 4 + 1) * 128],
                                qT[p][base : base + 64, :],
                                start=True,
                                stop=True,
                            )
                        nc.scalar.activation(
                            expT[:, hh, 2 * jj2 : 2 * jj2 + 2, :],
                            sc[:],
                            AF.Exp,
                            scale=float(D) ** -0.5,
                        )

                    # v jh=0 chains with sc/exp of pairs 0-1 interleaved so
                    # attention is primed when the first-half v completes
                    PRIMED = 2
                    prime_units = [(p, hh, jj2) for p in range(PRIMED)
                                   for hh in range(2) for jj2 in range(4)]
                    # one prime chunk per chain after the first six: the
                    # early-chain v copies must not queue behind priming
                    # exps on ACT (exps wait on qT, which lands last)
                    ui = 0
                    for st in range(ST):
                        nc.gpsimd.memset(vA[st][:, :, D:DA], 16.0)
                        v_chain(st, 0, p1_vps)
                        if ui < len(prime_units):
                            sc_exp_chunk(*prime_units[ui])
                            ui += 1
                    while ui < len(prime_units):
                        sc_exp_chunk(*prime_units[ui])
                        ui += 1

                    # ============= phase 2: attention =============
                    def attn_acc(h, hh, expT, rb_sb):
                        """attn_mean: accB[:, j, :] += exp_h[:, j, :] * rb,
                        j-split DVE(0:WSPLIT)/Pool(WSPLIT:8)."""
                        rbb = rb_sb[:].rearrange("p (a t) -> p a t", a=1)
                        wd, wp_ = WSPLIT, ST - WSPLIT
                        if h == 0:
                            nc.vector.tensor_mul(
                                accB[:, 0:wd, :], expT[:, hh, 0:wd, :],
                                rbb.broadcast_to((128, wd, TN)),
                            )
                            nc.gpsimd.tensor_mul(
                                accB[:, wd:ST, :], expT[:, hh, wd:ST, :],
                                rbb.broadcast_to((128, wp_, TN)),
                            )
                        else:
                            nc.vector.tensor_mul(
                                expT[:, hh, 0:wd, :], expT[:, hh, 0:wd, :],
                                rbb.broadcast_to((128, wd, TN)),
                            )
                            nc.vector.tensor_add(
                                accB[:, 0:wd, :], accB[:, 0:wd, :],
                                expT[:, hh, 0:wd, :],
                            )
                            nc.gpsimd.tensor_mul(
                                expT[:, hh, wd:ST, :], expT[:, hh, wd:ST, :],
                                rbb.broadcast_to((128, wp_, TN)),
                            )
                            nc.gpsimd.tensor_add(
                                accB[:, wd:ST, :], accB[:, wd:ST, :],
                                expT[:, hh, wd:ST, :],
                            )

                    # software-pipelined by TWO heads: rb travels through a
                    # DRAM round-trip (SBUF row -> DRAM -> partition-bcast
                    # back, ~4.5us latency on otherwise-idle DMA engines),
                    # so head h's yT/attn_acc run during head h+2.  The
                    # last two heads broadcast via a K=1 PE matmul instead
                    # (~1us) so the out-projection isn't held up.  The
                    # v jh=1 chains ride along pairs 0-3 as PE filler
                    # (their outputs are first read at head 8).
                    def finish_head(fin):
                        h, hh, mt, base, expT, y_ps, rb_sb = fin
                        # yT[c-tile mt, base:base+64] = 16 * y_aug * rb
                        nc.vector.scalar_tensor_tensor(
                            yT[mt][base : base + 64, :],
                            y_ps[0:64, :],
                            16.0,
                            rb_sb[0:64, :],
                            ALU.mult,
                            ALU.mult,
                        )
                        attn_acc(h, hh, expT, rb_sb)

                    pending = []
                    chunk_q = [(p_, hh_, jj2_) for p_ in range(PRIMED, H // 2)
                               for hh_ in range(2) for jj2_ in range(4)]
                    ci = [0]
                    with tc.tile_pool(name="p2_y", bufs=2, space="PSUM") as p2_y:
                        for p in range(H // 2):
                            mt = p
                            for hh in range(2):
                                h = 2 * p + hh
                                base = 64 * hh
                                # finish head h-2 BEFORE y_ps(h) is
                                # allocated (bufs=2 WAR rotation)
                                if len(pending) >= 2:
                                    finish_head(pending.pop(0))
                                # emit this head's remaining sc chunks (two
                                # were pre-issued as lookahead last head)
                                while ci[0] < max(0, (h + 1 - 2 * PRIMED)) * 4:
                                    sc_exp_chunk(*chunk_q[ci[0]])
                                    ci[0] += 1
                                expT = exp_tiles[p]
                                y_ps = p2_y.tile([DA, TN], F32, tag="y")
                                for j in range(ST):
                                    nc.tensor.matmul(
                                        y_ps[:],
                                        vA[j][:, h, :],
                                        expT[:, hh, j, :],
                                        start=(j == 0),
                                        stop=(j == ST - 1),
                                    )
                                if p < 4:
                                    v_chain(2 * p + hh, 1, p1_vps)
                                elif hh == 0 and p == 4:
                                    # incremental out-proj chains m=0,1 on
                                    # the two now-idle v PSUM banks: their
                                    # k=p-4 step rides each later pair, so
                                    # only k>=4 remains after the last head
                                    op_ps = [
                                        p1_vps.tile([128, TN], F32,
                                                    tag="proj",
                                                    name=f"op{i}_ps")
                                        for i in range(2)
                                    ]
                                if p >= 4 and 2 * (p - 4) + hh <= 5:
                                    # both chains advance one k-step per
                                    # head: k=2(p-4)+hh needs yT[k], i.e.
                                    # head 2k+1 finished -- true for k<=5
                                    # under the depth-2 finish pipeline
                                    ok = 2 * (p - 4) + hh
                                    for i in range(2):
                                        nc.tensor.matmul(
                                            op_ps[i][:],
                                            wp_sb[:, ok,
                                                  i * 128 : (i + 1) * 128],
                                            yT[ok][:],
                                            start=(ok == 0),
                                            stop=False,
                                        )
                                # row 64 of y_ps is 16*Z -> rb = 1/(16Z)
                                recip = p2_rb.tile([1, TN], BF16, tag="recip")
                                with nc.allow_low_precision(reason="softmax 1/Z"):
                                    nc.vector.reciprocal(recip[:], y_ps[64:65, :])
                                rb_sb = p2_rb.tile([128, TN], BF16, tag="rbsb")
                                if h < H - 2:
                                    rb_d = rbd.tile([TN], BF16, tag="rbd")
                                    nc.sync.dma_start(rb_d[:], recip[:])
                                    nc.sync.dma_start(
                                        rb_sb[:],
                                        rb_d[:].partition_broadcast(128),
                                    )
                                else:
                                    rb_ps = p2_sc.tile([128, 2, TN], F32,
                                                       tag="sc",
                                                       name=f"rbps{hh}")
                                    nc.tensor.matmul(
                                        rb_ps[:, 0, :], ones_row[:], recip[:],
                                        start=True, stop=True,
                                    )
                                    nc.scalar.activation(
                                        rb_sb[:], rb_ps[:, 0, :], AF.Copy
                                    )
                                pending.append(
                                    (h, hh, mt, base, expT, y_ps, rb_sb)
                                )
                                # 2-chunk lookahead into the next head: its
                                # 4 matmuls fit the 4-deep PE wait queue,
                                # so the exps cook during this head's tail
                                la = min(len(chunk_q),
                                         max(0, (h + 2 - 2 * PRIMED)) * 4 - 2)
                                while ci[0] < la:
                                    sc_exp_chunk(*chunk_q[ci[0]])
                                    ci[0] += 1
                        while pending:
                            finish_head(pending.pop(0))
                        nc.sync.dma_start(
                            am_o.ap().rearrange("(j p) t -> p j t", p=128),
                            accB[:],
                        )

                        # ===== phase 3: output projection =====
                        # emitted inside the attention pool scope (pool
                        # close would drain-barrier on the final attn_acc)
                        # and accumulated in the sc PSUM pool's banks.
                        with tc.tile_pool(name="p3_o", bufs=3) as p3_o:
                            # finish the incremental m=0,1 chains (k>=6)
                            for k in range(6, KT):
                                for i in range(2):
                                    nc.tensor.matmul(
                                        op_ps[i][:],
                                        wp_sb[:, k, i * 128 : (i + 1) * 128],
                                        yT[k][:],
                                        start=False,
                                        stop=(k == KT - 1),
                                    )
                            for i in range(2):
                                yo = p3_o.tile([128, TN], BF16, tag="yo_sb")
                                nc.scalar.activation(
                                    yo[:], op_ps[i][:], AF.Identity,
                                    bias=bp_sb[:, i : i + 1],
                                )
                                nc.sync.dma_start(
                                    yT_o.ap()[i * 128 : (i + 1) * 128, :],
                                    yo[:],
                                )
                            for m in range(2, MT):
                                ps = p2_sc.tile([128, TN], F32, tag="sc",
                                                name=f"yo_ps{m}")
                                for k in range(KT):
                                    nc.tensor.matmul(
                                        ps[:],
                                        wp_sb[:, k, m * 128 : (m + 1) * 128],
                                        yT[k][:],
                                        start=(k == 0),
                                        stop=(k == KT - 1),
                                    )
                                yo = p3_o.tile([128, TN], BF16, tag="yo_sb")
                                # bias add on ACT
                                nc.scalar.activation(
                                    yo[:], ps[:], AF.Identity,
                                    bias=bp_sb[:, m : m + 1],
                                )
                                nc.sync.dma_start(
                                    yT_o.ap()[m * 128 : (m + 1) * 128, :],
                                    yo[:],
                                )

    nc.compile()
    return nc


_NC_CACHE = None


def _get_nc():
    global _NC_CACHE
    if _NC_CACHE is None:
        _NC_CACHE = build()
    return _NC_CACHE


BF = ml_dtypes.bfloat16


def make_in_maps(x, encoder_output, Wq, bq, Wk, bk, Wv, bv, q_scale, k_scale,
                 Wp, bp):
    x = np.asarray(x, np.float32)
    enc = np.asarray(encoder_output, np.float32)
    Wk_b = np.asarray(Wk, np.float32).astype(BF)
    # m-sliced wk layout: wkr[p, m, k, c] = Wk[k*128+p, m*128+c]
    wkr = np.ascontiguousarray(
        Wk_b.reshape(KT, 128, MT, 128).transpose(1, 2, 0, 3)
    )
    Wq_b = np.ascontiguousarray(np.asarray(Wq, np.float32)).astype(BF)
    Wv_b = np.ascontiguousarray(np.asarray(Wv, np.float32)).astype(BF)
    Wp_b = np.ascontiguousarray(np.asarray(Wp, np.float32)).astype(BF)
    bq_t = np.ascontiguousarray(np.asarray(bq, np.float32).reshape(MT, 128).T)
    bk_t = np.ascontiguousarray(np.asarray(bk, np.float32).reshape(MT, 128).T)
    # bv folded into the out-proj bias: attn@(v+bv) = attn@v + bv
    bp_f = (np.asarray(bv, np.float32) @ np.asarray(Wp, np.float32)
            + np.asarray(bp, np.float32))
    bp_t = np.ascontiguousarray(bp_f.reshape(MT, 128).T)
    qs_t = np.ascontiguousarray(
        np.asarray(q_scale, np.float32).reshape(MT, 128).T
    )
    ks_t = np.ascontiguousarray(
        np.asarray(k_scale, np.float32).reshape(MT, 128).T
    )

    encT_b = [np.ascontiguousarray(enc[b].T).astype(BF) for b in range(B)]

    in_maps = []
    for c in range(8):
        b, hf = c // 2, c % 2
        xT = np.ascontiguousarray(x[b, hf * TN : (hf + 1) * TN, :].T).astype(BF)
        in_maps.append(
            dict(xT=xT, encT=encT_b[b], wkr=wkr, wq=Wq_b, wv=Wv_b, wp=Wp_b,
                 bq=bq_t, bk=bk_t, bp=bp_t, qs=qs_t, ks=ks_t)
        )
    return in_maps


def kernel(x, encoder_output, Wq, bq, Wk, bk, Wv, bv, q_scale, k_scale, Wp, bp,
           _trace=False):
    in_maps = make_in_maps(x, encoder_output, Wq, bq, Wk, bk, Wv, bv, q_scale,
                           k_scale, Wp, bp)
    nc = _get_nc()
    res = run_bass_kernel_spmd(nc, in_maps, core_ids=list(range(8)), trace=_trace)

    y = np.empty((B, T, C), np.float32)
    amean = np.empty((B, T, S), np.float32)
    for c in range(8):
        b, hf = c // 2, c % 2
        r = res.results[c]
        y[b, hf * TN : (hf + 1) * TN, :] = r["youtT"].astype(np.float32).T
        amean[b, hf * TN : (hf + 1) * TN, :] = (
            r["ameanT"].astype(np.float32).T
        )
    if _trace:
        kernel.last_exec_time_ns = res.exec_time_ns
        kernel.last_results = res
    return y, amean


# revision 72
# speedup vs baseline: 1.0053x; 1.0003x over previous
"""Cross-attention (B=4, T=S=1024, C=1024, H=16, D=64) on 8 trn2 NeuronCores.

Sharding: core c handles batch b=c//2, query half hf=c%2 (512 q-rows).
NO collectives: each core computes the FULL k/v for its batch (the extra
PE time replaces the ~225us AllGather the collective cost model charges).
All activations stay channel-major on chip; the host transposes per-core
inputs/outputs.

All matmuls run in bf16 (PSUM accumulation fp32); elementwise bf16 on DVE
gets the 2x packed mode.  PE executes only the 640 core GEMM matmuls
(k 128, q 64, v 128, scores 128, y 128, out-proj 64) plus 2 K=1 rb
broadcasts for the last two heads; every other auxiliary op is off PE:

  * RMSNorm sum-of-squares: ACT Square per m-tile, running adds on Pool
    (idle in phase 1), then ONE gpsimd partition_all_reduce -> ssum
    broadcast [128,t]; ACT sqrt and a lazily-emitted DVE reciprocal run
    at full partition width so rr needs no further broadcast (the old
    revision's 24 ones-matmuls + 24 K=1 outer products are gone).
  * RMSNorm apply: f = rr*scale + 1 (DVE two-scalar tensor_scalar),
    out = f*raw (DVE).
  * v bias: folded into the out-proj bias on the HOST (softmax rows sum
    to 1, so attn@(v+bv) = attn@v + bv -> bp' = bv@Wp + bp); the v-chain
    PSUM->SBUF move is a bare ACT copy.
  * softmax 1/Z: the v-augmentation column is memset to 16.0 so row 64
    of y_aug is 16*Z and the DVE reciprocal yields rb=1/(16Z) directly.
    rb is partition-broadcast via a DRAM round-trip (SBUF row -> DRAM ->
    partition_broadcast DMA back, ~4.5us on otherwise-idle DMA engines,
    hazard-tracked by a DRAM tile pool); heads 14/15 use a K=1 ones-row
    matmul + ACT copy instead so the out-projection isn't held up.
  * attn_mean accB[s,j,t] += exp*rb, j-split DVE(0:WSPLIT)/Pool(WSPLIT:8),
    software-pipelined by TWO heads to ride out the rb DMA latency
    (y PSUM bufs=2 tolerates this: y_ps(h) frees mid-iteration h+2).

Schedule: the attention window is elementwise-bound (~4.7us/head: DVE
accB+recip+yT ~4.6, ACT 4 exp chunks ~4.2, Pool accB ~4.3), so PE filler
rides inside it: the v jh=1 chains on head pairs 0-3 and two incremental
out-proj chains (m=0,1, one k-step per head from pair 4) on the v PSUM
banks, leaving a 52-matmul out-proj tail.  Priming (sc/exp of pairs 0-1)
interleaves with the v jh=0 chains.  The first k projection emits its
first six m-chains k-major across 6 PSUM banks so the per-k encoder DMAs
keep PE saturated from ~3.5us; wk arrives m-sliced ([128,m,k,128] host
layout) and small bias/scale DMAs ride between the e-tile DMAs.

TimelineSim 172631 ns (baseline 181514).  Measured rel err ~1.2e-2 vs
the fp32 reference (limit 2e-2).
"""

import numpy as np
import ml_dtypes

import concourse.bacc as bacc
import concourse.bass_isa as bass_isa
import concourse.mybir as mybir
import concourse.tile as tile
from concourse.bass_utils import run_bass_kernel_spmd

F32 = mybir.dt.float32
F32R = mybir.dt.float32r
BF16 = mybir.dt.bfloat16
AF = mybir.ActivationFunctionType
ALU = mybir.AluOpType
RED = bass_isa.ReduceOp

B, T, S, C, H = 4, 1024, 1024, 1024, 16
D = C // H            # 64
TN = 512              # per-core q rows
KT = 8                # contraction tiles (C/128)
MT = 8                # output-channel tiles
ST = 8                # s tiles (S/128)
DA = D + 1            # augmented head width (16.0 column at 64)
WSPLIT = 6            # attn-mean j-tiles on DVE (rest on Pool)


def build():
    nc = bacc.Bacc("TRN2", target_bir_lowering=False, debug=False, num_devices=8)

    xT_d = nc.dram_tensor("xT", [C, TN], BF16, kind="ExternalInput")
    encT_d = nc.dram_tensor("encT", [C, S], BF16, kind="ExternalInput")
    wkr_d = nc.dram_tensor("wkr", [128, MT, KT, 128], BF16, kind="ExternalInput")
    wq_d = nc.dram_tensor("wq", [C, C], BF16, kind="ExternalInput")
    wv_d = nc.dram_tensor("wv", [C, C], BF16, kind="ExternalInput")
    wp_d = nc.dram_tensor("wp", [C, C], BF16, kind="ExternalInput")
    bq_d = nc.dram_tensor("bq", [128, MT], F32, kind="ExternalInput")
    bk_d = nc.dram_tensor("bk", [128, MT], F32, kind="ExternalInput")
    bp_d = nc.dram_tensor("bp", [128, MT], F32, kind="ExternalInput")
    qs_d = nc.dram_tensor("qs", [128, MT], F32, kind="ExternalInput")
    ks_d = nc.dram_tensor("ks", [128, MT], F32, kind="ExternalInput")

    yT_o = nc.dram_tensor("youtT", [C, TN], BF16, kind="ExternalOutput")
    am_o = nc.dram_tensor("ameanT", [S, TN], BF16, kind="ExternalOutput")

    with tile.TileContext(nc) as tc:
        with (
            tc.tile_pool(name="const", bufs=1) as cst,
            tc.tile_pool(name="wk", bufs=1) as wk_pool,
            tc.tile_pool(name="qt", bufs=1) as qt_pool,
            tc.tile_pool(name="kt", bufs=1) as kt_pool,
            tc.tile_pool(name="vt", bufs=1) as vt_pool,
            tc.tile_pool(name="yt", bufs=1) as yt_pool,
            tc.tile_pool(name="am", bufs=1) as am_pool,
            tc.tile_pool(name="wp", bufs=1) as wp_pool,
        ):
            eps_t = cst.tile([128, 1], F32)
            nc.vector.memset(eps_t[:], 1e-6)
            ones_row = cst.tile([1, 128], BF16)
            nc.vector.memset(ones_row[:], 1.0)

            bq_sb = cst.tile([128, MT], F32)
            bk_sb = cst.tile([128, MT], F32)
            bp_sb = cst.tile([128, MT], F32)
            qs_sb = cst.tile([128, MT], F32)
            ks_sb = cst.tile([128, MT], F32)

            wk_sb = wk_pool.tile([128, MT, KT, 128], BF16, name="wk_sb")
            qT = [qt_pool.tile([128, TN], BF16, tag=f"qT{m}", name=f"qT{m}")
                  for m in range(MT)]
            kT = [kt_pool.tile([128, 2, TN], BF16, tag=f"kT{m}", name=f"kT{m}")
                  for m in range(MT)]
            vA = [vt_pool.tile([128, H, DA], BF16, tag=f"vA{j}", name=f"vA{j}")
                  for j in range(ST)]
            yT = [yt_pool.tile([128, TN], BF16, tag=f"yT{k}", name=f"yT{k}")
                  for k in range(KT)]
            accB = am_pool.tile([128, ST, TN], BF16, name="accB")
            wp_sb = wp_pool.tile([128, KT, C], BF16, name="wp_sb")

            # ================= phase 1: projections =================
            with (
                tc.tile_pool(name="p1_in", bufs=1) as p1_in,
                tc.tile_pool(name="p1_w", bufs=2) as p1_w,
            ):
                e_sb = p1_in.tile([128, KT, 2, TN], BF16, name="e_sb")
                x_sb = p1_in.tile([128, KT, TN], BF16, name="x_sb")

                # DMA order = issue order.  wk m=0 slice (729ns) + e0 gate
                # the first matmul at ~3us; the small bias/scale DMAs ride
                # between the e-tile transfers (HWDGE overlaps the bus).
                def e_dma(k, hf2):
                    nc.sync.dma_start(
                        e_sb[:, k, hf2, :],
                        encT_d.ap()[k * 128 : (k + 1) * 128,
                                    hf2 * TN : (hf2 + 1) * TN],
                    )

                # the k0 projection reads only the hf=0 half of each e
                # tile, so the halves load separately: hf0 k-tiles arrive
                # at twice the rate during the paced k-major phase
                # first k-slice of wk m0 alone (182ns) so the very
                # first matmul's inputs land as early as possible
                nc.sync.dma_start(wk_sb[:, 0, 0], wkr_d.ap()[:, 0, 0])
                e_dma(0, 0)
                nc.sync.dma_start(wk_sb[:, 0, 1:KT], wkr_d.ap()[:, 0, 1:KT])
                small = [(bk_sb, bk_d), (ks_sb, ks_d), (bq_sb, bq_d),
                         (qs_sb, qs_d), (bp_sb, bp_d)]
                for k in range(1, KT):
                    # wk m-slices ride ahead of the e tiles so the k-major
                    # interleaved chains m0-5 have weights from ~4us
                    if k < 6:
                        nc.sync.dma_start(wk_sb[:, k], wkr_d.ap()[:, k])
                    e_dma(k, 0)
                    if small:
                        sb, d = small.pop(0)
                        nc.sync.dma_start(sb[:], d.ap())
                for m in range(6, MT):
                    nc.sync.dma_start(wk_sb[:, m], wkr_d.ap()[:, m])
                for k in range(KT):
                    e_dma(k, 1)
                nc.sync.dma_start(
                    x_sb[:], xT_d.ap().rearrange("(k p) t -> p k t", p=128)
                )
                wq_sb = p1_w.tile([128, KT, C], BF16, tag="W", name="wq_sb")
                nc.sync.dma_start(
                    wq_sb[:], wq_d.ap().rearrange("(k p) c -> p k c", p=128)
                )
                wv_sb = p1_w.tile([128, KT, C], BF16, tag="W", name="wv_sb")
                nc.sync.dma_start(
                    wv_sb[:], wv_d.ap().rearrange("(k p) c -> p k c", p=128)
                )
                nc.sync.dma_start(
                    wp_sb[:], wp_d.ap().rearrange("(k p) c -> p k c", p=128)
                )

                def proj_tail(ps, m, bias_sb, acc, raws):
                    """Bias->raw (DVE), square (ACT), running sum (Pool)."""
                    raw = p1_raw.tile([128, TN], BF16, tag=f"raw{m}")
                    nc.vector.tensor_scalar_add(
                        raw[:], ps[:], bias_sb[:, m : m + 1]
                    )
                    sq = p1_sq.tile([128, TN], F32R, tag="sq")
                    nc.scalar.activation(
                        sq[:], ps[:], AF.Square, bias=bias_sb[:, m : m + 1]
                    )
                    raws.append(raw)
                    if m == 0:
                        proj_tail.sq0 = sq
                    elif m == 1:
                        nc.gpsimd.tensor_add(acc[:], proj_tail.sq0[:], sq[:])
                    else:
                        nc.gpsimd.tensor_add(acc[:], acc[:], sq[:])

                def proj_rms(acc):
                    """partition_all_reduce -> sqrt, both [128,TN].  The
                    DVE reciprocal is deferred to proj_B so it doesn't
                    head-block the in-order DVE queue while sqrt is still
                    pending."""
                    ssb = p1_r.tile([128, TN], F32R, tag="ssb", bufs=1)
                    nc.gpsimd.partition_all_reduce(ssb[:], acc[:], 128, RED.add)
                    rms = p1_r.tile([128, TN], F32, tag="rms", bufs=2)
                    nc.scalar.activation(
                        rms[:], ssb[:], AF.Sqrt, scale=1.0 / C,
                        bias=eps_t[:, 0:1],
                    )
                    return rms

                def proj_A(w_ap, in_ap, bias_sb, paced=False):
                    """Projection matmuls + bias/sum-of-squares tail.

                    paced=True emits the first four m-chains k-major across
                    4 PSUM banks so per-k input DMAs keep PE saturated.
                    """
                    acc = p1_r.tile([128, TN], F32R, tag="acc")
                    raws = []
                    if paced:
                        for lo, n in ((0, 6), (6, 2)):
                            pss = [p1_ps.tile([128, TN], F32, tag="proj",
                                              name=f"pacedps{i}")
                                   for i in range(n)]
                            for k in range(KT):
                                for i, ps in enumerate(pss):
                                    nc.tensor.matmul(
                                        ps[:],
                                        w_ap(lo + i, k),
                                        in_ap(k),
                                        start=(k == 0),
                                        stop=(k == KT - 1),
                                    )
                            for i, ps in enumerate(pss):
                                proj_tail(ps, lo + i, bias_sb, acc, raws)
                    else:
                        for m in range(MT):
                            ps = p1_ps.tile([128, TN], F32, tag="proj")
                            for k in range(KT):
                                nc.tensor.matmul(
                                    ps[:],
                                    w_ap(m, k),
                                    in_ap(k),
                                    start=(k == 0),
                                    stop=(k == KT - 1),
                                )
                            proj_tail(ps, m, bias_sb, acc, raws)
                    return raws, proj_rms(acc)

                def proj_B(raws, rms, scale_sb, out_ap, ms=None):
                    """RMSNorm+residual apply: out = (scale*rr + 1) * raw.

                    All ops on DVE (two-scalar tensor_scalar for f) —
                    ACT's in-order queue is the scarce phase-1 resource
                    (squares + v copies + priming exps).
                    """
                    rrB = p1_r.tile([128, TN], F32R, tag="rr")
                    with nc.allow_low_precision(reason="rms rsqrt"):
                        nc.vector.reciprocal(rrB[:], rms[:])
                    for m in (range(MT) if ms is None else ms):
                        f = p1_r.tile([128, TN], F32R, tag="f")
                        nc.vector.tensor_scalar(
                            f[:], rrB[:], scale_sb[:, m : m + 1], 1.0,
                            ALU.mult, ALU.add,
                        )
                        nc.vector.tensor_mul(out_ap(m), f[:], raws[m][:])

                def v_chain(st, jh, pool):
                    # bv is folded into the out-proj bias on the host
                    # (softmax rows sum to 1: attn@(v+bv) = attn@v + bv),
                    # so the PSUM->SBUF move is a pure ACT copy and the
                    # in-order DVE queue never gates the v chains.
                    ps = pool.tile([128, TN], F32, tag="proj")
                    for k in range(KT):
                        nc.tensor.matmul(
                            ps[:],
                            e_sb[:, k, st // 4,
                                 (st % 4) * 128 : (st % 4 + 1) * 128],
                            wv_sb[:, k, jh * 512 : (jh + 1) * 512],
                            start=(k == 0),
                            stop=(k == KT - 1),
                        )
                    nc.scalar.copy(
                        vA[st][:, jh * 8 : (jh + 1) * 8, 0:D],
                        ps[:].rearrange("p (h d) -> p h d", h=8),
                    )

                # qk projections; the jh=0 v chains share the ps pool and
                # carry the sc/exp priming of head pairs 0-1.
                with (
                    tc.tile_pool(name="p1_raw", bufs=2) as p1_raw,
                    tc.tile_pool(name="p1_sq", bufs=2) as p1_sq,
                    tc.tile_pool(name="p1_r", bufs=2) as p1_r,
                    tc.tile_pool(name="p1_ps", bufs=6, space="PSUM") as p1_ps,
                ):
                    k_w = lambda m, k: wk_sb[:, m, k, :]
                    k_in = [lambda k, hf2=hf2: e_sb[:, k, hf2, :]
                            for hf2 in range(2)]
                    k_out = [lambda m, hf2=hf2: kT[m][:, hf2, :]
                             for hf2 in range(2)]
                    q_w = lambda m, k: wq_sb[:, k, m * 128 : (m + 1) * 128]
                    rk0 = proj_A(k_w, k_in[0], bk_sb, paced=True)
                    rk1 = proj_A(k_w, k_in[1], bk_sb)
                    proj_B(*rk0, ks_sb, k_out[0])
                    rq = proj_A(q_w, lambda k: x_sb[:, k, :], bq_sb)
                    # qB ahead of kB1: the jj2<2 priming chunks read only
                    # the hf0 half of kT, so qT is the scarcer input
                    proj_B(*rq, qs_sb, lambda m: qT[m][:, :])
                    proj_B(*rk1, ks_sb, k_out[1])

                # score PSUM pool spans v-proj + attention so the sc/exp
                # chain of early head-pairs overlaps the v matmuls
                with (
                    tc.tile_pool(name="p2_e", bufs=3) as p2_e,
                    tc.tile_pool(name="p2_rb", bufs=3) as p2_rb,
                    tc.tile_pool(name="rbd", bufs=3, space="DRAM") as rbd,
                    tc.tile_pool(name="p2_sc", bufs=2, space="PSUM") as p2_sc,
                    tc.tile_pool(name="p1_vps", bufs=2, space="PSUM") as p1_vps,
                ):
                    exp_tiles = {}

                    def sc_exp_chunk(p, hh, jj2):
                        """Two K=64 score matmuls + one 2-bank exp."""
                        if p not in exp_tiles:
                            exp_tiles[p] = p2_e.tile(
                                [128, 2, ST, TN], BF16, tag="exp",
                                name=f"expT{p}",
                            )
                        expT = exp_tiles[p]
                        base = 64 * hh
                        sc = p2_sc.tile([128, 2, TN], F32, tag="sc")
                        for u in range(2):
                            j = 2 * jj2 + u
                            nc.tensor.matmul(
                                sc[:, u, :],
                                kT[p][base : base + 64, j // 4,
                                      (j % 4) * 128 : (j % 4 + 1) * 128],
                                qT[p][base : base + 64, :],
                                start=True,
                                stop=True,
                            )
                        nc.scalar.activation(
                            expT[:, hh, 2 * jj2 : 2 * jj2 + 2, :],
                            sc[:],
                            AF.Exp,
                            scale=float(D) ** -0.5,
                        )

                    # v jh=0 chains with sc/exp of pairs 0-1 interleaved so
                    # attention is primed when the first-half v completes
                    PRIMED = 2
                    prime_units = [(p, hh, jj2) for p in range(PRIMED)
                                   for hh in range(2) for jj2 in range(4)]
                    # one prime chunk per chain after the first six: the
                    # early-chain v copies must not queue behind priming
                    # exps on ACT (exps wait on qT, which lands last)
                    ui = 0
                    for st in range(ST):
                        nc.gpsimd.memset(vA[st][:, :, D:DA], 16.0)
                        v_chain(st, 0, p1_vps)
                        if ui < len(prime_units):
                            sc_exp_chunk(*prime_units[ui])
                            ui += 1
                    while ui < len(prime_units):
                        sc_exp_chunk(*prime_units[ui])
                        ui += 1

                    # ============= phase 2: attention =============
                    def attn_acc(h, hh, expT, rb_sb):
                        """attn_mean: accB[:, j, :] += exp_h[:, j, :] * rb,
                        j-split DVE(0:WSPLIT)/Pool(WSPLIT:8)."""
                        rbb = rb_sb[:].rearrange("p (a t) -> p a t", a=1)
                        wd, wp_ = WSPLIT, ST - WSPLIT
                        if h == 0:
                            nc.vector.tensor_mul(
                                accB[:, 0:wd, :], expT[:, hh, 0:wd, :],
                                rbb.broadcast_to((128, wd, TN)),
                            )
                            nc.gpsimd.tensor_mul(
                                accB[:, wd:ST, :], expT[:, hh, wd:ST, :],
                                rbb.broadcast_to((128, wp_, TN)),
                            )
                        else:
                            nc.vector.tensor_mul(
                                expT[:, hh, 0:wd, :], expT[:, hh, 0:wd, :],
                                rbb.broadcast_to((128, wd, TN)),
                            )
                            nc.vector.tensor_add(
                                accB[:, 0:wd, :], accB[:, 0:wd, :],
                                expT[:, hh, 0:wd, :],
                            )
                            nc.gpsimd.tensor_mul(
                                expT[:, hh, wd:ST, :], expT[:, hh, wd:ST, :],
                                rbb.broadcast_to((128, wp_, TN)),
                            )
                            nc.gpsimd.tensor_add(
                                accB[:, wd:ST, :], accB[:, wd:ST, :],
                                expT[:, hh, wd:ST, :],
                            )

                    # software-pipelined by TWO heads: rb travels through a
                    # DRAM round-trip (SBUF row -> DRAM -> partition-bcast
                    # back, ~4.5us latency on otherwise-idle DMA engines),
                    # so head h's yT/attn_acc run during head h+2.  The
                    # last two heads broadcast via a K=1 PE matmul instead
                    # (~1us) so the out-projection isn't held up.  The
                    # v jh=1 chains ride along pairs 0-3 as PE filler
                    # (their outputs are first read at head 8).
                    def finish_head(fin):
                        h, hh, mt, base, expT, y_ps, rb_sb = fin
                        # yT[c-tile mt, base:base+64] = 16 * y_aug * rb
                        nc.vector.scalar_tensor_tensor(
                            yT[mt][base : base + 64, :],
                            y_ps[0:64, :],
                            16.0,
                            rb_sb[0:64, :],
                            ALU.mult,
                            ALU.mult,
                        )
                        attn_acc(h, hh, expT, rb_sb)

                    pending = []
                    chunk_q = [(p_, hh_, jj2_) for p_ in range(PRIMED, H // 2)
                               for hh_ in range(2) for jj2_ in range(4)]
                    ci = [0]
                    with tc.tile_pool(name="p2_y", bufs=2, space="PSUM") as p2_y:
                        for p in range(H // 2):
                            mt = p
                            for hh in range(2):
                                h = 2 * p + hh
                                base = 64 * hh
                                # finish head h-2 BEFORE y_ps(h) is
                                # allocated (bufs=2 WAR rotation)
                                if len(pending) >= 2:
                                    finish_head(pending.pop(0))
                                # emit this head's remaining sc chunks (two
                                # were pre-issued as lookahead last head)
                                while ci[0] < max(0, (h + 1 - 2 * PRIMED)) * 4:
                                    sc_exp_chunk(*chunk_q[ci[0]])
                                    ci[0] += 1
                                expT = exp_tiles[p]
                                y_ps = p2_y.tile([DA, TN], F32, tag="y")
                                for j in range(ST):
                                    nc.tensor.matmul(
                                        y_ps[:],
                                        vA[j][:, h, :],
                                        expT[:, hh, j, :],
                                        start=(j == 0),
                                        stop=(j == ST - 1),
                                    )
                                if p < 4:
                                    v_chain(2 * p + hh, 1, p1_vps)
                                elif hh == 0 and p == 4:
                                    # incremental out-proj chains m=0,1 on
                                    # the two now-idle v PSUM banks: their
                                    # k=p-4 step rides each later pair, so
                                    # only k>=4 remains after the last head
                                    op_ps = [
                                        p1_vps.tile([128, TN], F32,
                                                    tag="proj",
                                                    name=f"op{i}_ps")
                                        for i in range(2)
                                    ]
                                if p >= 4 and 2 * (p - 4) + hh <= 5:
                                    # both chains advance one k-step per
                                    # head: k=2(p-4)+hh needs yT[k], i.e.
                                    # head 2k+1 finished -- true for k<=5
                                    # under the depth-2 finish pipeline
                                    ok = 2 * (p - 4) + hh
                                    for i in range(2):
                                        nc.tensor.matmul(
                                            op_ps[i][:],
                                            wp_sb[:, ok,
                                                  i * 128 : (i + 1) * 128],
                                            yT[ok][:],
                                            start=(ok == 0),
                                            stop=False,
                                        )
                                # row 64 of y_ps is 16*Z -> rb = 1/(16Z)
                                recip = p2_rb.tile([1, TN], BF16, tag="recip")
                                with nc.allow_low_precision(reason="softmax 1/Z"):
                                    nc.vector.reciprocal(recip[:], y_ps[64:65, :])
                                rb_sb = p2_rb.tile([128, TN], BF16, tag="rbsb")
                                if h < H - 2:
                                    rb_d = rbd.tile([TN], BF16, tag="rbd")
                                    nc.sync.dma_start(rb_d[:], recip[:])
                                    nc.sync.dma_start(
                                        rb_sb[:],
                                        rb_d[:].partition_broadcast(128),
                                    )
                                else:
                                    rb_ps = p2_sc.tile([128, 2, TN], F32,
                                                       tag="sc",
                                                       name=f"rbps{hh}")
                                    nc.tensor.matmul(
                                        rb_ps[:, 0, :], ones_row[:], recip[:],
                                        start=True, stop=True,
                                    )
                                    nc.scalar.activation(
                                        rb_sb[:], rb_ps[:, 0, :], AF.Copy
                                    )
                                pending.append(
                                    (h, hh, mt, base, expT, y_ps, rb_sb)
                                )
                                # 2-chunk lookahead into the next head: its
                                # 4 matmuls fit the 4-deep PE wait queue,
                                # so the exps cook during this head's tail
                                la = min(len(chunk_q),
                                         max(0, (h + 2 - 2 * PRIMED)) * 4 - 2)
                                while ci[0] < la:
                                    sc_exp_chunk(*chunk_q[ci[0]])
                                    ci[0] += 1
                        while pending:
                            finish_head(pending.pop(0))
                        nc.sync.dma_start(
                            am_o.ap().rearrange("(j p) t -> p j t", p=128),
                            accB[:],
                        )

                        # ===== phase 3: output projection =====
                        # emitted inside the attention pool scope (pool
                        # close would drain-barrier on the final attn_acc)
                        # and accumulated in the sc PSUM pool's banks.
                        with tc.tile_pool(name="p3_o", bufs=3) as p3_o:
                            # finish the incremental m=0,1 chains (k>=6)
                            for k in range(6, KT):
                                for i in range(2):
                                    nc.tensor.matmul(
                                        op_ps[i][:],
                                        wp_sb[:, k, i * 128 : (i + 1) * 128],
                                        yT[k][:],
                                        start=False,
                                        stop=(k == KT - 1),
                                    )
                            for i in range(2):
                                yo = p3_o.tile([128, TN], BF16, tag="yo_sb")
                                nc.scalar.activation(
                                    yo[:], op_ps[i][:], AF.Identity,
                                    bias=bp_sb[:, i : i + 1],
                                )
                                nc.sync.dma_start(
                                    yT_o.ap()[i * 128 : (i + 1) * 128, :],
                                    yo[:],
                                )
                            for m in range(2, MT):
                                ps = p2_sc.tile([128, TN], F32, tag="sc",
                                                name=f"yo_ps{m}")
                                for k in range(KT):
                                    nc.tensor.matmul(
                                        ps[:],
                                        wp_sb[:, k, m * 128 : (m + 1) * 128],
                                        yT[k][:],
                                        start=(k == 0),
                                        stop=(k == KT - 1),
                                    )
                                yo = p3_o.tile([128, TN], BF16, tag="yo_sb")
                                # bias add on ACT
                                nc.scalar.activation(
                                    yo[:], ps[:], AF.Identity,
                                    bias=bp_sb[:, m : m + 1],
                                )
                                nc.sync.dma_start(
                                    yT_o.ap()[m * 128 : (m + 1) * 128, :],
                                    yo[:],
                                )

    nc.compile()
    return nc


_NC_CACHE = None


def _get_nc():
    global _NC_CACHE
    if _NC_CACHE is None:
        _NC_CACHE = build()
    return _NC_CACHE


BF = ml_dtypes.bfloat16


def make_in_maps(x, encoder_output, Wq, bq, Wk, bk, Wv, bv, q_scale, k_scale,
                 Wp, bp):
    x = np.asarray(x, np.float32)
    enc = np.asarray(encoder_output, np.float32)
    Wk_b = np.asarray(Wk, np.float32).astype(BF)
    # m-sliced wk layout: wkr[p, m, k, c] = Wk[k*128+p, m*128+c]
    wkr = np.ascontiguousarray(
        Wk_b.reshape(KT, 128, MT, 128).transpose(1, 2, 0, 3)
    )
    Wq_b = np.ascontiguousarray(np.asarray(Wq, np.float32)).astype(BF)
    Wv_b = np.ascontiguousarray(np.asarray(Wv, np.float32)).astype(BF)
    Wp_b = np.ascontiguousarray(np.asarray(Wp, np.float32)).astype(BF)
    bq_t = np.ascontiguousarray(np.asarray(bq, np.float32).reshape(MT, 128).T)
    bk_t = np.ascontiguousarray(np.asarray(bk, np.float32).reshape(MT, 128).T)
    # bv folded into the out-proj bias: attn@(v+bv) = attn@v + bv
    bp_f = (np.asarray(bv, np.float32) @ np.asarray(Wp, np.float32)
            + np.asarray(bp, np.float32))
    bp_t = np.ascontiguousarray(bp_f.reshape(MT, 128).T)
    qs_t = np.ascontiguousarray(
        np.asarray(q_scale, np.float32).reshape(MT, 128).T
    )
    ks_t = np.ascontiguousarray(
        np.asarray(k_scale, np.float32).reshape(MT, 128).T
    )

    encT_b = [np.ascontiguousarray(enc[b].T).astype(BF) for b in range(B)]

    in_maps = []
    for c in range(8):
        b, hf = c // 2, c % 2
        xT = np.ascontiguousarray(x[b, hf * TN : (hf + 1) * TN, :].T).astype(BF)
        in_maps.append(
            dict(xT=xT, encT=encT_b[b], wkr=wkr, wq=Wq_b, wv=Wv_b, wp=Wp_b,
                 bq=bq_t, bk=bk_t, bp=bp_t, qs=qs_t, ks=ks_t)
        )
    return in_maps


def kernel(x, encoder_output, Wq, bq, Wk, bk, Wv, bv, q_scale, k_scale, Wp, bp,
           _trace=False):
    in_maps = make_in_maps(x, encoder_output, Wq, bq, Wk, bk, Wv, bv, q_scale,
                           k_scale, Wp, bp)
    nc = _get_nc()
    res = run_bass_kernel_spmd(nc, in_maps, core_ids=list(range(8)), trace=_trace)

    y = np.empty((B, T, C), np.float32)
    amean = np.empty((B, T, S), np.float32)
    for c in range(8):
        b, hf = c // 2, c % 2
        r = res.results[c]
        y[b, hf * TN : (hf + 1) * TN, :] = r["youtT"].astype(np.float32).T
        amean[b, hf * TN : (hf + 1) * TN, :] = (
            r["ameanT"].astype(np.float32).T
        )
    if _trace:
        kernel.last_exec_time_ns = res.exec_time_ns
        kernel.last_results = res
    return y, amean
